# revision 26
# baseline (speedup 1.0000x reference)
"""Trainium2 Bass kernel for CustomConv1d.

Problem: y = conv1d(x, weight, bias), x [32, 256, 4096] f32,
weight [256, 256, 5] f32, bias [256] f32, stride 1, pad 2.

Strategy: data-parallel over batch across 8 NeuronCores (4 batches/core,
weights+bias broadcast, no collectives). Per core the conv is computed as
matmuls on the tensor engine: for each output-channel chunk (128) and each
512-wide output tile, accumulate 10 matmuls in PSUM (5 taps x 2 input-channel
chunks of 128):

  out[co, w] = sum_{k, ci} weight[co, ci, k] * xpad[ci, w + k]

with lhsT = weight slice [ci(128 part), co(128)] and rhs = x slice
[ci(128 part), 512], both tagged float32r (full-rate fp32 matmul, ~1e-4
rel err). x arrives host-padded ([.., W+4]) so every tap is a plain
contiguous slice and no memset is needed (memset can't write f32r).
"""

import os

import numpy as np

import concourse.bass as bass
import concourse.bass_utils as bass_utils
import concourse.mybir as mybir
import concourse.tile as tile
from concourse import bacc
from concourse.bass_utils import run_bass_kernel_spmd


BF16 = mybir.dt.bfloat16

B, CIN, COUT, W, K, PAD = 32, 256, 256, 4096, 5, 2
NCORES = 8
BPC = B // NCORES          # batches per core
P = 128                    # partition dim
NT = 512                   # moving-operand tile (one fp32 PSUM bank)
N_CIC = CIN // P           # input-channel chunks
N_COC = COUT // P          # output-channel chunks
N_WT = W // NT             # output width tiles
WPADDED = W + 2 * PAD
ST = 2 * NT                # output store chunk (overlap tail stores)

F32 = mybir.dt.float32
F32R = mybir.dt.float32r


def _build_program():
    # Bacc (not plain Bass): its finalize() runs generate_event_semaphores,
    # which splits multi-sem waits into event-semaphore chains — the TRN2
    # walrus here accepts at most one sync wait per regular instruction.
    nc = bacc.Bacc()
    # x arrives host-padded: x[b, ci, :] = [0, 0, x_orig, 0, 0] (WPADDED cols)
    x_d = nc.declare_dram_parameter("x", [BPC, CIN, WPADDED], F32, isOutput=False)
    wt_d = nc.declare_dram_parameter("wt", [K, CIN, COUT], F32, isOutput=False)
    b_d = nc.declare_dram_parameter("bias2", [P, N_COC], F32, isOutput=False)
    o_d = nc.declare_dram_parameter("out", [BPC, COUT, W], F32, isOutput=True)

    with tile.TileContext(nc) as tc:
        with (
            tc.tile_pool(name="wpool", bufs=1) as wpool,
            tc.tile_pool(name="xpool", bufs=2 * N_CIC) as xpool,
            tc.tile_pool(name="opool", bufs=2 * N_COC) as opool,
            tc.tile_pool(name="psum", bufs=8, space="PSUM") as pspool,
        ):
            # PE warm-up scratch: memset early on DVE, dummy bf16 matmuls
            # below keep the HAM clock-gate busy while x/w stream in, so the
            # real matmul stream starts at 2.4 GHz instead of 1.2 GHz.
            warm = wpool.tile([P, NT], BF16)
            nc.vector.memset(warm[:], 0.0)

            # Weights: one strided DMA, issued first (single ~0.8us issue):
            # w_sb[ci, k, cic, co_full] = weight[co_full, cic*P+ci, k]
            w_sb = wpool.tile([P, K, N_CIC, COUT], F32R)
            w_src = wt_d[:].rearrange("k (cic ci) co -> ci k cic co", ci=P)
            nc.sync.dma_start(w_sb[:], w_src.bitcast(F32R))

            # First batch's x: 4 separate halo tiles per cic (Tile dep
            # tracking is per-tile, so a single chunked tile would gate every
            # matmul on the LAST chunk's DMA). Chunk c holds padded cols
            # [c*2*NT, c*2*NT + 2*NT + 2*PAD) = groups n=2c, 2c+1.
            CWH = 2 * NT + 2 * PAD
            x0c = []  # [cic][c] -> tile
            for cic in range(N_CIC):
                x0c.append([])
                for c in range(N_WT // 2):
                    t = xpool.tile(
                        [P, CWH], F32R, tag="xc", bufs=N_CIC * N_WT // 2,
                        name=f"x0_{cic}_{c}",
                    )
                    x0c[cic].append(t)
            for c in range(N_WT // 2):
                for cic in range(N_CIC):
                    nc.sync.dma_start(
                        x0c[cic][c][:],
                        x_d[0, cic * P:(cic + 1) * P, c * 2 * NT:c * 2 * NT + CWH]
                        .bitcast(F32R),
                    )

            # bias2 host-transposed to [P, N_COC] -> single [128, 2] DMA
            b_sb = wpool.tile([P, N_COC], F32)
            nc.sync.dma_start(b_sb[:], b_d[:])

            ps_warm = pspool.tile([P, NT], F32, tag="ps", name="ps_warm")
            for _ in range(7):
                nc.tensor.matmul(ps_warm[:], warm[:, 0:P], warm[:])

            for b in range(BPC):
                if b > 0:
                    xts = []
                    for cic in range(N_CIC):
                        xt = xpool.tile([P, WPADDED], F32R, tag="x", name=f"x{b}_{cic}")
                        nc.sync.dma_start(
                            xt[:], x_d[b, cic * P:(cic + 1) * P, :].bitcast(F32R)
                        )
                        xts.append(xt)

                last_pass = b == BPC - 1
                for coc in range(N_COC):
                    ot = opool.tile([P, W], F32, tag="o")
                    st = NT if (last_pass and coc == N_COC - 1) else ST
                    for n in range(N_WT):
                        ps = pspool.tile([P, NT], F32, tag="ps", name=f"ps{b}_{coc}_{n}")
                        idx = 0
                        for k in range(K):
                            for cic in range(N_CIC):
                                if b == 0:
                                    rhs = x0c[cic][n // 2][
                                        :, (n % 2) * NT + k:(n % 2) * NT + k + NT
                                    ]
                                else:
                                    rhs = xts[cic][:, n * NT + k:n * NT + k + NT]
                                nc.tensor.matmul(
                                    ps[:],
                                    w_sb[:, k, cic, coc * P:(coc + 1) * P],
                                    rhs,
                                    start=(idx == 0),
                                    stop=(idx == K * N_CIC - 1),
                                )
                                idx += 1
                        nc.vector.tensor_scalar_add(
                            ot[:, n * NT:(n + 1) * NT], ps[:], b_sb[:, coc:coc + 1]
                        )
                        # store as soon as a full chunk of st cols is ready
                        if ((n + 1) * NT) % st == 0:
                            c0 = (n + 1) * NT - st
                            nc.sync.dma_start(
                                o_d[b, coc * P:(coc + 1) * P, c0:c0 + st],
                                ot[:, c0:c0 + st],
                            )
    nc.finalize()
    return nc


_NC_CACHE = []


def kernel(x, weight, bias):
    assert x.shape == (B, CIN, W) and weight.shape == (COUT, CIN, K)
    if not _NC_CACHE:
        _NC_CACHE.append(_build_program())
    nc = _NC_CACHE[0]

    # wt[k, ci, co] = weight[co, ci, k]
    wt = np.ascontiguousarray(weight.astype(np.float32).transpose(2, 1, 0))
    bias2 = np.ascontiguousarray(bias.astype(np.float32).reshape(N_COC, P).T)
    xpad = np.pad(x.astype(np.float32), ((0, 0), (0, 0), (PAD, PAD)))
    in_maps = [
        {
            "x": np.ascontiguousarray(xpad[i * BPC:(i + 1) * BPC]),
            "wt": wt,
            "bias2": bias2,
        }
        for i in range(NCORES)
    ]
    res = run_bass_kernel_spmd(
        nc,
        in_maps,
        list(range(NCORES)),
        trace=bool(int(os.environ.get("KERNEL_TRACE", "0"))),
    )
    kernel.last_results = res
    return np.concatenate([res.results[i]["out"] for i in range(NCORES)], axis=0)


# revision 29
# speedup vs baseline: 1.0078x; 1.0078x over previous
"""Trainium2 Bass kernel for CustomConv1d.

Problem: y = conv1d(x, weight, bias), x [32, 256, 4096] f32,
weight [256, 256, 5] f32, bias [256] f32, stride 1, pad 2.

Strategy: data-parallel over batch across 8 NeuronCores (4 batches/core,
weights+bias broadcast, no collectives). Per core the conv is computed as
matmuls on the tensor engine: for each output-channel chunk (128) and each
512-wide output tile, accumulate 10 matmuls in PSUM (5 taps x 2 input-channel
chunks of 128):

  out[co, w] = sum_{k, ci} weight[co, ci, k] * xpad[ci, w + k]

with lhsT = weight slice [ci(128 part), co(128)] and rhs = x slice
[ci(128 part), 512], both tagged float32r (full-rate fp32 matmul, ~1e-4
rel err). x arrives host-padded ([.., W+4]) so every tap is a plain
contiguous slice and no memset is needed (memset can't write f32r).
"""

import os

import numpy as np

import concourse.bass as bass
import concourse.bass_utils as bass_utils
import concourse.mybir as mybir
import concourse.tile as tile
from concourse import bacc
from concourse.bass_utils import run_bass_kernel_spmd


BF16 = mybir.dt.bfloat16

B, CIN, COUT, W, K, PAD = 32, 256, 256, 4096, 5, 2
NCORES = 8
BPC = B // NCORES          # batches per core
P = 128                    # partition dim
NT = 512                   # moving-operand tile (one fp32 PSUM bank)
N_CIC = CIN // P           # input-channel chunks
N_COC = COUT // P          # output-channel chunks
N_WT = W // NT             # output width tiles
WPADDED = W + 2 * PAD
ST = 2 * NT                # output store chunk (overlap tail stores)

F32 = mybir.dt.float32
F32R = mybir.dt.float32r


def _build_program():
    # Bacc (not plain Bass): its finalize() runs generate_event_semaphores,
    # which splits multi-sem waits into event-semaphore chains — the TRN2
    # walrus here accepts at most one sync wait per regular instruction.
    nc = bacc.Bacc()
    # x arrives host-padded: x[b, ci, :] = [0, 0, x_orig, 0, 0] (WPADDED cols)
    x_d = nc.declare_dram_parameter("x", [BPC, CIN, WPADDED], F32, isOutput=False)
    # weights arrive host-transposed in the exact SBUF layout so the load is
    # one DMA of 128 contiguous 10KB lines (a strided load of the same data
    # costs ~12us of per-descriptor processing)
    wt_d = nc.declare_dram_parameter("wt", [P, K, N_CIC, COUT], F32, isOutput=False)
    b_d = nc.declare_dram_parameter("bias2", [P, N_COC], F32, isOutput=False)
    o_d = nc.declare_dram_parameter("out", [BPC, COUT, W], F32, isOutput=True)

    with tile.TileContext(nc) as tc:
        with (
            tc.tile_pool(name="wpool", bufs=1) as wpool,
            tc.tile_pool(name="xpool", bufs=2 * N_CIC) as xpool,
            tc.tile_pool(name="opool", bufs=2 * N_COC) as opool,
            tc.tile_pool(name="psum", bufs=8, space="PSUM") as pspool,
        ):
            # PE warm-up scratch: memset early on DVE, dummy bf16 matmuls
            # below keep the HAM clock-gate busy while x/w stream in, so the
            # real matmul stream starts at 2.4 GHz instead of 1.2 GHz.
            warm = wpool.tile([P, NT], BF16)
            nc.vector.memset(warm[:], 0.0)

            # Weights: one contiguous DMA, issued first:
            # w_sb[ci, k, cic, co_full] = weight[co_full, cic*P+ci, k]
            w_sb = wpool.tile([P, K, N_CIC, COUT], F32R)
            nc.sync.dma_start(w_sb[:], wt_d[:].bitcast(F32R))

            # First batch's x: 4 separate halo tiles per cic (Tile dep
            # tracking is per-tile, so a single chunked tile would gate every
            # matmul on the LAST chunk's DMA). Chunk c holds padded cols
            # [c*2*NT, c*2*NT + 2*NT + 2*PAD) = groups n=2c, 2c+1.
            CWH = 2 * NT + 2 * PAD
            x0c = []  # [cic][c] -> tile
            for cic in range(N_CIC):
                x0c.append([])
                for c in range(N_WT // 2):
                    t = xpool.tile(
                        [P, CWH], F32R, tag="xc", bufs=N_CIC * N_WT // 2,
                        name=f"x0_{cic}_{c}",
                    )
                    x0c[cic].append(t)
            for c in range(N_WT // 2):
                for cic in range(N_CIC):
                    nc.sync.dma_start(
                        x0c[cic][c][:],
                        x_d[0, cic * P:(cic + 1) * P, c * 2 * NT:c * 2 * NT + CWH]
                        .bitcast(F32R),
                    )

            # bias2 host-transposed to [P, N_COC] -> single [128, 2] DMA
            b_sb = wpool.tile([P, N_COC], F32)
            nc.sync.dma_start(b_sb[:], b_d[:])

            ps_warm = pspool.tile([P, NT], F32, tag="ps", name="ps_warm")
            for _ in range(7):
                nc.tensor.matmul(ps_warm[:], warm[:, 0:P], warm[:])

            for b in range(BPC):
                if b > 0:
                    xts = []
                    for cic in range(N_CIC):
                        xt = xpool.tile([P, WPADDED], F32R, tag="x", name=f"x{b}_{cic}")
                        nc.sync.dma_start(
                            xt[:], x_d[b, cic * P:(cic + 1) * P, :].bitcast(F32R)
                        )
                        xts.append(xt)

                last_pass = b == BPC - 1
                for coc in range(N_COC):
                    ot = opool.tile([P, W], F32, tag="o")
                    st = NT if (last_pass and coc == N_COC - 1) else ST
                    for n in range(N_WT):
                        ps = pspool.tile([P, NT], F32, tag="ps", name=f"ps{b}_{coc}_{n}")
                        idx = 0
                        for k in range(K):
                            for cic in range(N_CIC):
                                if b == 0:
                                    rhs = x0c[cic][n // 2][
                                        :, (n % 2) * NT + k:(n % 2) * NT + k + NT
                                    ]
                                else:
                                    rhs = xts[cic][:, n * NT + k:n * NT + k + NT]
                                nc.tensor.matmul(
                                    ps[:],
                                    w_sb[:, k, cic, coc * P:(coc + 1) * P],
                                    rhs,
                                    start=(idx == 0),
                                    stop=(idx == K * N_CIC - 1),
                                )
                                idx += 1
                        nc.vector.tensor_scalar_add(
                            ot[:, n * NT:(n + 1) * NT], ps[:], b_sb[:, coc:coc + 1]
                        )
                        # store as soon as a full chunk of st cols is ready
                        if ((n + 1) * NT) % st == 0:
                            c0 = (n + 1) * NT - st
                            nc.sync.dma_start(
                                o_d[b, coc * P:(coc + 1) * P, c0:c0 + st],
                                ot[:, c0:c0 + st],
                            )
    nc.finalize()
    return nc


_NC_CACHE = []


def kernel(x, weight, bias):
    assert x.shape == (B, CIN, W) and weight.shape == (COUT, CIN, K)
    if not _NC_CACHE:
        _NC_CACHE.append(_build_program())
    nc = _NC_CACHE[0]

    # wt[ci, k, cic, co] = weight[co, cic*128+ci, k]  (SBUF layout)
    wt = np.ascontiguousarray(
        weight.astype(np.float32)
        .transpose(1, 2, 0)               # [ci_full, k, co]
        .reshape(N_CIC, P, K, COUT)       # [cic, ci, k, co]
        .transpose(1, 2, 0, 3)            # [ci, k, cic, co]
    )
    bias2 = np.ascontiguousarray(bias.astype(np.float32).reshape(N_COC, P).T)
    xpad = np.pad(x.astype(np.float32), ((0, 0), (0, 0), (PAD, PAD)))
    in_maps = [
        {
            "x": np.ascontiguousarray(xpad[i * BPC:(i + 1) * BPC]),
            "wt": wt,
            "bias2": bias2,
        }
        for i in range(NCORES)
    ]
    res = run_bass_kernel_spmd(
        nc,
        in_maps,
        list(range(NCORES)),
        trace=bool(int(os.environ.get("KERNEL_TRACE", "0"))),
    )
    kernel.last_results = res
    return np.concatenate([res.results[i]["out"] for i in range(NCORES)], axis=0)
